# revision 1
# baseline (speedup 1.0000x reference)
"""APPNP+BN GNN kernel for 8 Trainium2 NeuronCores (Bass).

Sharding: nodes are degree-sorted and dealt round-robin to 8 cores (node/data
parallel).  Each core owns 6250 destination rows (padded to 6272 = 49 chunks
of 128).  Per propagate step: every core's pre-scaled activation shard is
AllGathered into a full node table in DRAM; each core gathers its edges'
source rows with indirect DMA ([128,1] per-partition row gather), segment-sums
them on the Vector engine (fixed slot grid: chunk c has k_c slot columns =
max degree in chunk), and applies the D^-1/2 scalings, biases, BN (batch
stats via ones-matmul partition reduction + tiny AllReduce), ReLU and weight
matmuls (TensorE, via identity transposes).  Output is log_softmax rows which
the host inverse-permutes.
"""

import contextlib
import numpy as np

import concourse.bacc as bacc
import concourse.mybir as mybir
from concourse.bass import AP, IndirectOffsetOnAxis
from concourse.bass_utils import run_bass_kernel_spmd

N = 50000
E = 800000
INF = 128
HID = 64
NCLS = 64
NCORES = 8
PAD_N = 6272          # 49 * 128 rows per core
NCHUNK = 49
REAL = 6250
TAB = PAD_N * NCORES  # 50176 table rows
ZROW = 6250           # core 0's first pad row: always zero post-AllGather
ALPHA = 0.1
K_STEPS = 10
BN_EPS = 1e-5
F32 = mybir.dt.float32


def _plan(edge_index):
    src = np.asarray(edge_index[0], dtype=np.int64)
    dst = np.asarray(edge_index[1], dtype=np.int64)
    loop = np.arange(N, dtype=np.int64)
    src = np.concatenate([src, loop])
    dst = np.concatenate([dst, loop])
    deg = np.bincount(dst, minlength=N)  # >= 1 everywhere (self loops)

    order = np.argsort(-deg, kind="stable")          # rank -> node
    ranks = np.empty(N, np.int64)
    ranks[order] = np.arange(N)
    core_of = (ranks % NCORES).astype(np.int64)       # node -> core
    lr_of = (ranks // NCORES).astype(np.int64)        # node -> local rank
    pi = core_of * PAD_N + lr_of                      # node -> table row

    pc_deg = np.ones((NCORES, PAD_N), np.float32)
    pc_deg[core_of, lr_of] = deg

    kc = np.zeros(NCHUNK, np.int64)
    for ch in range(NCHUNK):
        kc[ch] = int(pc_deg[:, ch * 128:(ch + 1) * 128].max())
    K = int(kc.sum())
    coloff = np.concatenate([[0], np.cumsum(kc)]).astype(np.int64)

    gidx = np.full((NCORES, 128, K), ZROW, np.int32)
    s_row = pi[src]
    d_core = core_of[dst]
    d_lr = lr_of[dst]
    for c in range(NCORES):
        m = d_core == c
        ls = d_lr[m]
        sr = s_row[m]
        o = np.argsort(ls, kind="stable")
        ls = ls[o]
        sr = sr[o]
        cnt = np.bincount(ls, minlength=PAD_N)
        off = np.concatenate([[0], np.cumsum(cnt)])
        t = np.arange(ls.size) - off[ls]
        p = ls % 128
        ch = ls // 128
        col = coloff[ch] + t
        gidx[c, p, col] = sr

    deg_pc = np.ones((NCORES, 128, NCHUNK), np.float32)
    for c in range(NCORES):
        deg_pc[c] = pc_deg[c].reshape(NCHUNK, 128).T

    return dict(order=order, kc=kc, K=K, coloff=coloff, gidx=gidx,
                deg_pc=deg_pc, core_of=core_of, lr_of=lr_of)


def _bcast_f(vec2d, nf=HID):
    """[128, C] sbuf AP -> [128, C, nf] with feature step 0 (free broadcast)."""
    b = vec2d
    return AP(b.tensor, b.offset, list(b.ap) + [[0, nf]])


def _bcast_col(vec2d_col, nf=HID):
    """[128, 1] slice -> [128, nf] with step 0."""
    b = vec2d_col
    return AP(b.tensor, b.offset, [b.ap[0], [0, nf]])


def _bcast_rep(rep, nch=NCHUNK):
    """[128, nf] replicated vec -> [128, nch, nf] with chunk step 0."""
    b = rep
    return AP(b.tensor, b.offset, [b.ap[0], [0, nch], b.ap[1]])


def _perm_kf(gb, k):
    """gbuf[:, 0:k, :] ([128, k, 64]) -> AP ordered [p, f, k] so the
    innermost (reduced) axis is the slot axis."""
    b = gb[:, 0:k, :]
    return AP(b.tensor, b.offset, [b.ap[0], [1, HID], [HID, k]])


def _build(kc, coloff, K):
    kc = [int(x) for x in kc]
    KMAX = max(max(kc), NCHUNK)
    nc = bacc.Bacc(target_bir_lowering=False)

    xs = nc.declare_dram_parameter("xs", [PAD_N, INF], F32, isOutput=False)
    gx = nc.declare_dram_parameter("gidx", [128, K], mybir.dt.int32, isOutput=False)
    dgp = nc.declare_dram_parameter("degp", [128, NCHUNK], F32, isOutput=False)
    idq = nc.declare_dram_parameter("ident", [128, 128], F32, isOutput=False)
    onc = nc.declare_dram_parameter("onescol", [128, 1], F32, isOutput=False)
    onr = nc.declare_dram_parameter("onesrow", [1, 128], F32, isOutput=False)
    pmk = nc.declare_dram_parameter("padmask", [128, 1], F32, isOutput=False)
    wts = {}
    for nm, shp in [("W1", [INF, HID]), ("W2", [HID, HID]), ("WX0", [HID, HID]),
                    ("WX1", [HID, HID]), ("Wfc", [HID, NCLS])]:
        wts[nm] = nc.declare_dram_parameter(nm, shp, F32, isOutput=False)
    vecs = {}
    for nm in ["b1", "b2", "bx0", "bx1", "bfc", "g1", "be1", "g2", "be2", "g3", "be3"]:
        vecs[nm] = nc.declare_dram_parameter(nm, [1, HID], F32, isOutput=False)
    out_d = nc.declare_dram_parameter("out", [REAL, NCLS], F32, isOutput=True)

    bounce_a = nc.dram_tensor("bounce_a", [PAD_N, HID], F32)
    bounce_b = nc.dram_tensor("bounce_b", [PAD_N, HID], F32)
    table = nc.dram_tensor("table", [TAB, HID], F32, addr_space="Shared")
    stat_i = nc.dram_tensor("stat_i", [1, 2 * HID], F32)
    stat_o = nc.dram_tensor("stat_o", [1, 2 * HID], F32, addr_space="Shared")

    ctx = contextlib.ExitStack()
    sb = lambda name, shp, dt=F32: ctx.enter_context(nc.sbuf_tensor(name, shp, dt))
    ps = lambda name, shp: ctx.enter_context(nc.psum_tensor(name, shp, F32))
    sem = lambda name: ctx.enter_context(nc.semaphore(name))

    with ctx:
        s_x = sb("s_x", [128, NCHUNK, INF])
        s_gx = sb("s_gx", [128, K], mybir.dt.int32)
        s_dg = sb("s_dg", [128, NCHUNK])
        s_id = sb("s_id", [128, 128])
        s_onc = sb("s_onc", [128, 1])
        s_onr = sb("s_onr", [1, 128])
        s_pmk = sb("s_pmk", [128, 1])
        s_w1 = sb("s_w1", [INF, HID])
        s_wsq = {nm: sb("s_" + nm, [HID, HID]) for nm in ["W2", "WX0", "WX1", "Wfc"]}
        s_vec = {nm: sb("sv_" + nm, [1, HID]) for nm in
                 ["b1", "b2", "bx0", "bx1", "bfc", "g1", "be1", "g2", "be2", "g3", "be3"]}
        s_rep = {nm: sb("sr_" + nm, [128, HID]) for nm in
                 ["b1", "b2", "bx0", "bx1", "bfc", "a", "beta"]}
        dinv = sb("dinv", [128, NCHUNK])
        dsq = sb("dsq", [128, NCHUNK])
        d2s = sb("d2s", [128, NCHUNK])
        tmpc = sb("tmpc", [128, NCHUNK])
        zz = sb("zz", [128, NCHUNK, HID])
        uu = sb("uu", [128, NCHUNK, HID])
        w0 = sb("w0", [128, NCHUNK, HID])
        hh = sb("hh", [128, NCHUNK, HID])
        gbA = sb("gbA", [128, KMAX, HID])
        gbB = sb("gbB", [128, KMAX, HID])
        s_ht = sb("s_ht", [HID, 128])
        s_xt = sb("s_xt", [128, 128])
        sA = sb("sA", [128, HID])
        sB = sb("sB", [128, HID])
        s_st = sb("s_st", [1, 2 * HID])
        v1 = sb("v1", [1, HID])
        v2 = sb("v2", [1, HID])
        v3 = sb("v3", [1, HID])
        v4 = sb("v4", [1, HID])
        vmax = sb("vmax", [128, NCHUNK])
        pT = ps("pT", [128, 128])
        pM = ps("pM", [128, HID])
        pR = ps("pR", [128, HID])
        pS1 = ps("pS1", [1, HID])
        pS2 = ps("pS2", [1, HID])

        S = {k: sem("sem_" + k) for k in ["dma", "ind", "coll", "mm", "dve", "act", "dma2"]}
        C = {k: 0 for k in S}

        with nc.Block() as block:

            @block.gpsimd
            def _(g):
                V, T, A, Y = nc.vector, nc.tensor, nc.scalar, nc.sync

                def w(eng, *keys):
                    for k in keys:
                        eng.wait_ge(S[k], C[k])

                def dma(out, in_, eng=g):
                    eng.dma_start(out=out, in_=in_).then_inc(S["dma"], 16)
                    C["dma"] += 16

                def vop(fn, *a, **kw):
                    fn(*a, **kw).then_inc(S["dve"], 1)
                    C["dve"] += 1
                    V.wait_ge(S["dve"], C["dve"])

                def top(fn, *a, **kw):
                    fn(*a, **kw).then_inc(S["mm"], 1)
                    C["mm"] += 1
                    T.wait_ge(S["mm"], C["mm"])

                def aop(fn, *a, **kw):
                    fn(*a, **kw).then_inc(S["act"], 1)
                    C["act"] += 1
                    A.wait_ge(S["act"], C["act"])

                # ---- init loads ----
                dma(s_x[:, :, :], xs[:, :].rearrange("(c p) f -> p c f", p=128))
                dma(s_gx[:, :], gx[:, :])
                dma(s_dg[:, :], dgp[:, :])
                dma(s_id[:, :], idq[:, :])
                dma(s_onc[:, :], onc[:, :])
                dma(s_onr[:, :], onr[:, :])
                dma(s_pmk[:, :], pmk[:, :])
                dma(s_w1[:, :], wts["W1"][:, :])
                for nm in s_wsq:
                    dma(s_wsq[nm][:, :], wts[nm][:, :])
                for nm in s_vec:
                    dma(s_vec[nm][:, :], vecs[nm][:, :])

                # degree-derived vectors
                w(A, "dma")
                aop(A.activation, dsq[:, :], s_dg[:, :], mybir.ActivationFunctionType.Sqrt)
                w(V, "act")
                vop(V.reciprocal, dinv[:, :], dsq[:, :])
                vop(V.tensor_mul, tmpc[:, :], dinv[:, :], dinv[:, :])
                vop(V.tensor_scalar_mul, d2s[:, :], tmpc[:, :], 1.0 - ALPHA)

                # replicate bias vectors across partitions: ones_row.T @ vec
                w(T, "dma")
                for nm in ["b1", "b2", "bx0", "bx1", "bfc"]:
                    top(T.matmul, pR[:, :], s_onr[:, :], s_vec[nm][:, :], start=True, stop=True)
                    w(V, "mm")
                    vop(V.tensor_copy, s_rep[nm][:, :], pR[:, :])
                    w(T, "dve")

                def replicate(vec_ap, dst_rep):
                    w(T, "dve", "act")
                    top(T.matmul, pR[:, :], s_onr[:, :], vec_ap, start=True, stop=True)
                    w(V, "mm")
                    vop(V.tensor_copy, dst_rep[:, :], pR[:, :])
                    w(T, "dve")

                # layer-1 table: uu = dinv * (x @ W1)
                w(T, "dve")
                for c in range(NCHUNK):
                    top(T.transpose, pT[:, :], s_x[:, c, :], s_id[:, :])
                    w(V, "mm")
                    vop(V.tensor_copy, s_xt[:, :], pT[:, :])
                    w(T, "dve")
                    top(T.matmul, pM[:, :], s_xt[:, :], s_w1[:, :], start=True, stop=True)
                    w(V, "mm")
                    vop(V.tensor_tensor, uu[:, c, :], pM[:, :],
                        _bcast_col(dinv[:, c:c + 1]), op=mybir.AluOpType.mult)
                    w(T, "dve")

                bb = {"cur": bounce_a, "nxt": bounce_b}

                def propagate(update=False, prebounced=False):
                    """AllGather uu -> table; gather+segment-sum -> zz.

                    update: fold the APPNP u-update (uu = d2s*zz + w0) into the
                    per-chunk tail and bounce each finished chunk on the Sync
                    engine (HWDGE) into the OTHER bounce buffer (a peer may
                    still be pulling the current one for this AllGather).
                    prebounced: uu was already bounced by the previous
                    propagate's per-chunk Sync DMAs."""
                    bounce = bb["cur"]
                    if not prebounced:
                        # zero the pad rows, then bounce out serially
                        w(V, "ind")
                        vop(V.tensor_tensor, uu[:, 48, :], uu[:, 48, :],
                            _bcast_col(s_pmk[:, 0:1]), op=mybir.AluOpType.mult)
                        w(g, "dve")
                        dma(bounce[:, :].rearrange("(c p) f -> p c f", p=128), uu[:, :, :])
                        g.wait_ge(S["dma"], C["dma"])
                    else:
                        g.wait_ge(S["dma2"], C["dma2"])
                    g.wait_ge(S["ind"], C["ind"])
                    g.collective_compute(
                        "AllGather", mybir.AluOpType.bypass,
                        replica_groups=[list(range(NCORES))],
                        ins=[bounce.ap().opt()], outs=[table.ap().opt()],
                    ).then_inc(S["coll"], 1)
                    C["coll"] += 1
                    g.wait_ge(S["coll"], C["coll"])
                    V.wait_ge(S["dma2"], C["dma2"])  # uu WAR vs sync bounces
                    red_done = []
                    for c in range(NCHUNK):
                        gb = gbA if (c % 2 == 0) else gbB
                        if c >= 2:
                            g.wait_ge(S["dve"], red_done[c - 2])
                        for j in range(kc[c]):
                            col = int(coloff[c]) + j
                            g.indirect_dma_start(
                                out=gb[:, j, :], out_offset=None,
                                in_=table[:, :],
                                in_offset=IndirectOffsetOnAxis(ap=s_gx[:, col:col + 1], axis=0),
                            ).then_inc(S["ind"], 16)
                            C["ind"] += 16
                        V.wait_ge(S["ind"], C["ind"])
                        vop(V.tensor_reduce, zz[:, c, :], _perm_kf(gb, kc[c]),
                            mybir.AxisListType.X, mybir.AluOpType.add)
                        red_done.append(C["dve"])
                        if update:
                            vop(V.tensor_tensor, uu[:, c, :], zz[:, c, :],
                                _bcast_col(d2s[:, c:c + 1]), op=mybir.AluOpType.mult)
                            vop(V.tensor_add, uu[:, c, :], uu[:, c, :], w0[:, c, :])
                            Y.wait_ge(S["dve"], C["dve"])
                            Y.dma_start(out=bb["nxt"][128 * c:128 * (c + 1), :],
                                        in_=uu[:, c, :]).then_inc(S["dma2"], 16)
                            C["dma2"] += 16
                    if update:
                        bb["cur"], bb["nxt"] = bb["nxt"], bb["cur"]

                def matmul_layer(w_sb, dst, scale_vec):
                    """dst[:,c,:] = scale_vec * (hh @ W) ; scale_vec [128,NCHUNK]."""
                    w(T, "dve", "act")
                    for c in range(NCHUNK):
                        top(T.transpose, pT[0:HID, :], hh[:, c, :], s_id[:, :])
                        w(V, "mm")
                        vop(V.tensor_copy, s_ht[:, :], pT[0:HID, :])
                        w(T, "dve")
                        top(T.matmul, pM[:, :], s_ht[:, :], w_sb[:, :], start=True, stop=True)
                        w(V, "mm")
                        vop(V.tensor_tensor, dst[:, c, :], pM[:, :],
                            _bcast_col(scale_vec[:, c:c + 1]), op=mybir.AluOpType.mult)
                        w(T, "dve")

                def bn_relu(bias_nm, g_nm, be_nm):
                    """zz := relu(bn(dinv*zz + b)) -> hh (batch stats across cores)."""
                    w(V, "ind")
                    vop(V.tensor_tensor, zz[:, :, :], zz[:, :, :],
                        _bcast_f(dinv[:, :]), op=mybir.AluOpType.mult)
                    vop(V.tensor_tensor, zz[:, :, :], zz[:, :, :],
                        _bcast_rep(s_rep[bias_nm][:, :]), op=mybir.AluOpType.add)
                    # full sums incl. the 22 pad rows/core (pad row == bias
                    # vector exactly); corrected analytically after AllReduce
                    b = zz[:, :, :]
                    zzkf = AP(b.tensor, b.offset, [b.ap[0], [1, HID], [HID, NCHUNK]])
                    vop(V.tensor_reduce, sA[:, :], zzkf,
                        mybir.AxisListType.X, mybir.AluOpType.add)
                    zq = gbB
                    vop(V.tensor_mul, zq[:, 0:NCHUNK, :], zz[:, :, :], zz[:, :, :])
                    bq = zq[:, 0:NCHUNK, :]
                    zqkf = AP(bq.tensor, bq.offset, [bq.ap[0], [1, HID], [HID, NCHUNK]])
                    vop(V.tensor_reduce, sB[:, :], zqkf,
                        mybir.AxisListType.X, mybir.AluOpType.add)
                    w(T, "dve")
                    top(T.matmul, pS1[:, :], s_onc[:, :], sA[:, :], start=True, stop=True)
                    top(T.matmul, pS2[:, :], s_onc[:, :], sB[:, :], start=True, stop=True)
                    w(V, "mm")
                    vop(V.tensor_copy, s_st[0:1, 0:HID], pS1[:, :])
                    vop(V.tensor_copy, s_st[0:1, HID:2 * HID], pS2[:, :])
                    w(g, "dve")
                    dma(stat_i[:, :], s_st[:, :])
                    g.wait_ge(S["dma"], C["dma"])
                    g.collective_compute(
                        "AllReduce", mybir.AluOpType.add,
                        replica_groups=[list(range(NCORES))],
                        ins=[stat_i.ap().opt()], outs=[stat_o.ap().opt()],
                    ).then_inc(S["coll"], 1)
                    C["coll"] += 1
                    g.wait_ge(S["coll"], C["coll"])
                    dma(s_st[:, :], stat_o[:, :])
                    w(V, "dma")
                    npad = float(NCORES * (PAD_N - REAL))
                    vop(V.tensor_scalar_mul, v1[:, :], s_vec[bias_nm][:, :], npad)
                    vop(V.tensor_sub, v1[:, :], s_st[0:1, 0:HID], v1[:, :])
                    vop(V.tensor_scalar_mul, v1[:, :], v1[:, :], 1.0 / N)       # mean
                    vop(V.tensor_mul, v2[:, :], s_vec[bias_nm][:, :], s_vec[bias_nm][:, :])
                    vop(V.tensor_scalar_mul, v2[:, :], v2[:, :], npad)
                    vop(V.tensor_sub, v2[:, :], s_st[0:1, HID:2 * HID], v2[:, :])
                    vop(V.tensor_scalar_mul, v2[:, :], v2[:, :], 1.0 / N)       # E[x^2]
                    vop(V.tensor_mul, v3[:, :], v1[:, :], v1[:, :])
                    vop(V.tensor_sub, v2[:, :], v2[:, :], v3[:, :])             # var
                    vop(V.tensor_scalar_add, v2[:, :], v2[:, :], BN_EPS)
                    w(A, "dve")
                    aop(A.activation, v3[:, :], v2[:, :], mybir.ActivationFunctionType.Sqrt)
                    w(V, "act")
                    vop(V.reciprocal, v4[:, :], v3[:, :])                        # rstd
                    vop(V.tensor_mul, v4[:, :], v4[:, :], s_vec[g_nm][:, :])     # a
                    vop(V.tensor_mul, v3[:, :], v1[:, :], v4[:, :])
                    vop(V.tensor_sub, v3[:, :], s_vec[be_nm][:, :], v3[:, :])    # beta
                    replicate(v4[:, :], s_rep["a"])
                    replicate(v3[:, :], s_rep["beta"])
                    w(V, "dve")
                    vop(V.tensor_tensor, hh[:, :, :], zz[:, :, :],
                        _bcast_rep(s_rep["a"][:, :]), op=mybir.AluOpType.mult)
                    vop(V.tensor_tensor, hh[:, :, :], hh[:, :, :],
                        _bcast_rep(s_rep["beta"][:, :]), op=mybir.AluOpType.add)
                    vop(V.tensor_scalar_max, hh[:, :, :], hh[:, :, :], 0.0)

                # ---- 4 GCN layers ----
                layer_params = [("b1", "g1", "be1", "W2", "b2"),
                                ("b2", "g2", "be2", "WX0", "bx0"),
                                ("bx0", "g3", "be3", "WX1", "bx1"),
                                ("bx1", "g3", "be3", None, None)]
                for li, (bias_nm, g_nm, be_nm, next_w, _nb) in enumerate(layer_params):
                    propagate()
                    bn_relu(bias_nm, g_nm, be_nm)
                    if next_w is not None:
                        matmul_layer(s_wsq[next_w], uu, dinv)
                    else:
                        # APPNP setup: u0 = dinv*h0 ; w0 = alpha*u0
                        w(V, "ind")
                        vop(V.tensor_tensor, uu[:, :, :], hh[:, :, :],
                            _bcast_f(dinv[:, :]), op=mybir.AluOpType.mult)
                        vop(V.tensor_scalar_mul, w0[:, :, :], uu[:, :, :], ALPHA)
                        # mask pad rows so per-chunk updates keep uu pads zero
                        vop(V.tensor_tensor, d2s[:, 48:49], d2s[:, 48:49],
                            s_pmk[:, 0:1], op=mybir.AluOpType.mult)
                        vop(V.tensor_tensor, w0[:, 48, :], w0[:, 48, :],
                            _bcast_col(s_pmk[:, 0:1]), op=mybir.AluOpType.mult)

                # ---- APPNP power iterations ----
                for _k in range(K_STEPS):
                    propagate(update=True, prebounced=(_k > 0))

                # ---- final: h = uu * sqrt(deg); out = log_softmax(h @ Wfc + bfc) ----
                w(V, "ind")
                vop(V.tensor_tensor, hh[:, :, :], uu[:, :, :],
                    _bcast_f(dsq[:, :]), op=mybir.AluOpType.mult)
                w(T, "dve", "act")
                for c in range(NCHUNK):
                    top(T.transpose, pT[0:HID, :], hh[:, c, :], s_id[:, :])
                    w(V, "mm")
                    vop(V.tensor_copy, s_ht[:, :], pT[0:HID, :])
                    w(T, "dve")
                    top(T.matmul, pM[:, :], s_ht[:, :], s_wsq["Wfc"][:, :], start=True, stop=True)
                    w(V, "mm")
                    vop(V.tensor_tensor, zz[:, c, :], pM[:, :],
                        s_rep["bfc"][:, :], op=mybir.AluOpType.add)
                    w(T, "dve")
                # log_softmax over features (free axis)
                vop(V.tensor_reduce, vmax[:, :], zz[:, :, :],
                    mybir.AxisListType.X, mybir.AluOpType.max)
                vop(V.tensor_tensor, zz[:, :, :], zz[:, :, :],
                    _bcast_f(vmax[:, :]), op=mybir.AluOpType.subtract)
                ee = gbA
                w(A, "dve")
                aop(A.activation, ee[:, 0:NCHUNK, :], zz[:, :, :],
                    mybir.ActivationFunctionType.Exp)
                w(V, "act")
                vop(V.tensor_reduce, vmax[:, :], ee[:, 0:NCHUNK, :],
                    mybir.AxisListType.X, mybir.AluOpType.add)
                w(A, "dve")
                aop(A.activation, tmpc[:, :], vmax[:, :], mybir.ActivationFunctionType.Ln)
                w(V, "act")
                vop(V.tensor_tensor, zz[:, :, :], zz[:, :, :],
                    _bcast_f(tmpc[:, :]), op=mybir.AluOpType.subtract)
                w(g, "dve")
                dma(out_d[0:6144, :].rearrange("(c p) f -> p c f", p=128), zz[:, 0:48, :])
                dma(out_d[6144:REAL, :], zz[0:106, 48, :])
                g.wait_ge(S["dma"], C["dma"])

    nc.compile()
    return nc


def kernel(**inputs):
    x = np.asarray(inputs["x"], np.float32)
    plan = _plan(np.asarray(inputs["edge_index"]))
    nc = _build(plan["kc"], plan["coloff"], plan["K"])

    order, core_of, lr_of = plan["order"], plan["core_of"], plan["lr_of"]
    ident = np.eye(128, dtype=np.float32)
    onescol = np.ones((128, 1), np.float32)
    onesrow = np.ones((1, 128), np.float32)
    Wx = np.asarray(inputs["Wx"], np.float32)
    bx = np.asarray(inputs["bx"], np.float32)
    common = {
        "ident": ident, "onescol": onescol, "onesrow": onesrow,
        "padmask": (np.arange(128) < REAL - 48 * 128).astype(np.float32)[:, None],
        "W1": np.asarray(inputs["W1"], np.float32),
        "W2": np.asarray(inputs["W2"], np.float32),
        "WX0": Wx[0], "WX1": Wx[1],
        "Wfc": np.asarray(inputs["Wfc"], np.float32),
        "b1": np.asarray(inputs["b1"], np.float32)[None, :],
        "b2": np.asarray(inputs["b2"], np.float32)[None, :],
        "bx0": bx[0][None, :], "bx1": bx[1][None, :],
        "bfc": np.asarray(inputs["bfc"], np.float32)[None, :],
        "g1": np.asarray(inputs["g1"], np.float32)[None, :],
        "be1": np.asarray(inputs["be1"], np.float32)[None, :],
        "g2": np.asarray(inputs["g2"], np.float32)[None, :],
        "be2": np.asarray(inputs["be2"], np.float32)[None, :],
        "g3": np.asarray(inputs["g3"], np.float32)[None, :],
        "be3": np.asarray(inputs["be3"], np.float32)[None, :],
    }
    in_maps = []
    for c in range(NCORES):
        xs = np.zeros((PAD_N, INF), np.float32)
        nodes = order[c::NCORES]          # rank r*8+c -> local rank r
        xs[:nodes.size] = x[nodes]
        m = {"xs": xs,
             "gidx": plan["gidx"][c],
             "degp": plan["deg_pc"][c]}
        m.update(common)
        in_maps.append(m)

    res = run_bass_kernel_spmd(nc, in_maps, core_ids=list(range(NCORES)), trace=False)
    out = np.empty((N, NCLS), np.float32)
    for c in range(NCORES):
        nodes = order[c::NCORES]
        out[nodes] = res.results[c]["out"][:nodes.size]
    return out



# revision 3
# speedup vs baseline: 1.0148x; 1.0148x over previous
"""APPNP+BN GNN kernel for 8 Trainium2 NeuronCores (Bass).

Sharding: nodes are degree-sorted and dealt round-robin to 8 cores (node/data
parallel).  Each core owns 6250 destination rows (padded to 6272 = 49 chunks
of 128).  Per propagate step: every core's pre-scaled activation shard is
AllGathered into a full node table in DRAM; each core fetches its edges'
source rows with dma_gather (4 SWDGE queues, int16 indices, so the table is
addressed as two <32768-row halves; a host-side greedy pass balances each
destination row's sources across the halves), segment-sums them on the Vector
engine (fixed slot grid per half: chunk c has kA_c/kB_c slot columns = max
per-half degree in chunk), and applies the D^-1/2 scalings, biases, BN (batch
stats via ones-matmul partition reduction + tiny AllReduce), ReLU and weight
matmuls (TensorE, via identity transposes).  Output is log_softmax rows which
the host inverse-permutes.
"""

import contextlib
import numpy as np

import concourse.bacc as bacc
import concourse.mybir as mybir
from concourse.bass import AP
from concourse.bass_utils import run_bass_kernel_spmd
from concourse.library_config import mlp as _mlp_lib

N = 50000
E = 800000
INF = 128
HID = 64
NCLS = 64
NCORES = 8
PAD_N = 6272          # 49 * 128 rows per core
NCHUNK = 49
REAL = 6250
TAB = PAD_N * NCORES  # 50176 table rows
HALF = TAB // 2       # 25088: gather half A = rows [0, HALF), B = [HALF, TAB)
ZROW = 6250           # zero pad row, same local index in both halves
ALPHA = 0.1
K_STEPS = 10
BN_EPS = 1e-5
GMAX = 64             # max slot columns per dma_gather op
F32 = mybir.dt.float32


def _balance_bits(src, dst, ranks, deg):
    """Greedy per-node half-assignment: nodes with bit 0 land in table half A
    (cores 0-3), bit 1 in half B.  Exactly 512 of each per 1024-rank span so
    the round-robin core deal stays balanced.  Objective: for every dest node,
    split its in-edges' sources evenly between halves (keeps per-chunk max
    slot counts kA+kB close to kc)."""
    nspan = (N + 1023) // 1024
    capA = np.zeros(nspan, np.int64)
    capB = np.zeros(nspan, np.int64)
    for s in range(nspan):
        tot = min(1024, N - s * 1024)
        capA[s] = tot // 2
        capB[s] = tot - tot // 2
    # out-adjacency: for node n (as source), list of dests
    order_e = np.argsort(src, kind="stable")
    s_sorted = src[order_e]
    d_sorted = dst[order_e]
    starts = np.searchsorted(s_sorted, np.arange(N))
    ends = np.searchsorted(s_sorted, np.arange(N) + 1)
    cntA = np.zeros(N, np.int32)
    cntB = np.zeros(N, np.int32)
    bit = np.zeros(N, np.int8)
    # process in descending out-degree
    for n in np.argsort(-(ends - starts), kind="stable"):
        sp = ranks[n] // 1024
        ds = d_sorted[starts[n]:ends[n]]
        if capA[sp] == 0:
            b = 1
        elif capB[sp] == 0:
            b = 0
        else:
            # prefer the half where these dests currently have fewer sources
            b = 0 if int(cntA[ds].sum()) <= int(cntB[ds].sum()) else 1
        bit[n] = b
        if b == 0:
            capA[sp] -= 1
            cntA[ds] += 1
        else:
            capB[sp] -= 1
            cntB[ds] += 1
    return bit


def _plan(edge_index):
    src = np.asarray(edge_index[0], dtype=np.int64)
    dst = np.asarray(edge_index[1], dtype=np.int64)
    loop = np.arange(N, dtype=np.int64)
    src = np.concatenate([src, loop])
    dst = np.concatenate([dst, loop])
    deg = np.bincount(dst, minlength=N)  # >= 1 everywhere (self loops)

    order = np.argsort(-deg, kind="stable")          # rank -> node
    ranks = np.empty(N, np.int64)
    ranks[order] = np.arange(N)

    bit = _balance_bits(src, dst, ranks, deg)
    # within each 1024-rank span: bit-0 nodes -> cores 0-3, bit-1 -> cores 4-7
    core_of = np.empty(N, np.int64)
    lr_of = np.empty(N, np.int64)
    for s in range((N + 1023) // 1024):
        span = order[s * 1024:(s + 1) * 1024]        # nodes in rank order
        a = span[bit[span] == 0]
        b = span[bit[span] == 1]
        for g, grp in ((0, a), (4, b)):
            for i, n in enumerate(grp):
                core_of[n] = g + i % 4
                lr_of[n] = s * 128 + i // 4
    pi = core_of * PAD_N + lr_of                      # node -> table row

    pc_deg = np.ones((NCORES, PAD_N), np.float32)
    pc_deg[core_of, lr_of] = deg

    s_row = pi[src]
    d_core = core_of[dst]
    d_lr = lr_of[dst]
    half_of = (s_row >= HALF).astype(np.int64)

    # per-chunk per-half max slot counts (shared across cores: SPMD)
    kA = np.zeros(NCHUNK, np.int64)
    kB = np.zeros(NCHUNK, np.int64)
    cnt = np.zeros((2, NCORES, PAD_N), np.int64)
    np.add.at(cnt, (half_of, d_core, d_lr), 1)
    for ch in range(NCHUNK):
        kA[ch] = max(1, int(cnt[0, :, ch * 128:(ch + 1) * 128].max()))
        kB[ch] = max(1, int(cnt[1, :, ch * 128:(ch + 1) * 128].max()))
    KA = int(kA.sum())
    KB = int(kB.sum())
    aoff = np.concatenate([[0], np.cumsum(kA)]).astype(np.int64)
    boff = np.concatenate([[0], np.cumsum(kB)]).astype(np.int64)

    # chunk groups: consecutive chunks with sum kA <= GMAX and sum kB <= GMAX
    groups = []
    cur = [0]
    sa = int(kA[0])
    sb = int(kB[0])
    for ch in range(1, NCHUNK):
        if sa + kA[ch] <= GMAX and sb + kB[ch] <= GMAX:
            cur.append(ch)
            sa += int(kA[ch])
            sb += int(kB[ch])
        else:
            groups.append(cur)
            cur = [ch]
            sa = int(kA[ch])
            sb = int(kB[ch])
    groups.append(cur)

    # per-core index grids [128, K] (int16 local rows, pad = ZROW)
    gA = np.full((NCORES, 128, KA), ZROW, np.int16)
    gB = np.full((NCORES, 128, KB), ZROW, np.int16)
    for c in range(NCORES):
        for h, (gg, off) in enumerate(((gA, aoff), (gB, boff))):
            m = (d_core == c) & (half_of == h)
            ls = d_lr[m]
            sr = s_row[m] - h * HALF
            o = np.argsort(ls, kind="stable")
            ls = ls[o]
            sr = sr[o]
            bc = np.bincount(ls, minlength=PAD_N)
            eoff = np.concatenate([[0], np.cumsum(bc)])
            t = np.arange(ls.size) - eoff[ls]
            p = ls % 128
            ch = ls // 128
            col = off[ch] + t
            gg[c, p, col] = sr.astype(np.int16)

    # wrapped int16 index streams for dma_gather: index i = col*128 + p lives
    # at [i % 16 + 16*rep, i // 16]
    def wrap(gg, K):
        out = np.empty((NCORES, 128, K * 8), np.int16)
        for c in range(NCORES):
            flat = gg[c].T.reshape(-1)           # i = col*128 + p
            w = flat.reshape(-1, 16).T           # [16, K*8]
            out[c] = np.tile(w, (8, 1))
        return out

    deg_pc = np.ones((NCORES, 128, NCHUNK), np.float32)
    for c in range(NCORES):
        deg_pc[c] = pc_deg[c].reshape(NCHUNK, 128).T

    return dict(order=order, kA=kA, kB=kB, KA=KA, KB=KB, aoff=aoff, boff=boff,
                groups=groups, gAw=wrap(gA, KA), gBw=wrap(gB, KB),
                deg_pc=deg_pc, core_of=core_of, lr_of=lr_of)


def _bcast_f(vec2d, nf=HID):
    b = vec2d
    return AP(b.tensor, b.offset, list(b.ap) + [[0, nf]])


def _bcast_col(vec2d_col, nf=HID):
    b = vec2d_col
    return AP(b.tensor, b.offset, [b.ap[0], [0, nf]])


def _bcast_rep(rep, nch=NCHUNK):
    b = rep
    return AP(b.tensor, b.offset, [b.ap[0], [0, nch], b.ap[1]])


def _perm_kf(buf, c0, k):
    """buf[:, c0:c0+k, :] ([128, k, 64]) -> AP ordered [p, f, k] so the
    innermost (reduced) axis is the slot axis."""
    b = buf[:, c0:c0 + k, :]
    return AP(b.tensor, b.offset, [b.ap[0], [1, HID], [HID, k]])


def _build(plan):
    kA = [int(x) for x in plan["kA"]]
    kB = [int(x) for x in plan["kB"]]
    aoff = [int(x) for x in plan["aoff"]]
    boff = [int(x) for x in plan["boff"]]
    groups = plan["groups"]
    KA, KB = plan["KA"], plan["KB"]
    nc = bacc.Bacc(target_bir_lowering=False, num_swdge_queues=4)

    xs = nc.declare_dram_parameter("xs", [PAD_N, INF], F32, isOutput=False)
    gxa = nc.declare_dram_parameter("gxa", [128, KA * 8], mybir.dt.int16, isOutput=False)
    gxb = nc.declare_dram_parameter("gxb", [128, KB * 8], mybir.dt.int16, isOutput=False)
    dgp = nc.declare_dram_parameter("degp", [128, NCHUNK], F32, isOutput=False)
    idq = nc.declare_dram_parameter("ident", [128, 128], F32, isOutput=False)
    onc = nc.declare_dram_parameter("onescol", [128, 1], F32, isOutput=False)
    onr = nc.declare_dram_parameter("onesrow", [1, 128], F32, isOutput=False)
    pmk = nc.declare_dram_parameter("padmask", [128, 1], F32, isOutput=False)
    wts = {}
    for nm, shp in [("W1", [INF, HID]), ("W2", [HID, HID]), ("WX0", [HID, HID]),
                    ("WX1", [HID, HID]), ("Wfc", [HID, NCLS])]:
        wts[nm] = nc.declare_dram_parameter(nm, shp, F32, isOutput=False)
    vecs = {}
    for nm in ["b1", "b2", "bx0", "bx1", "bfc", "g1", "be1", "g2", "be2", "g3", "be3"]:
        vecs[nm] = nc.declare_dram_parameter(nm, [1, HID], F32, isOutput=False)
    out_d = nc.declare_dram_parameter("out", [REAL, NCLS], F32, isOutput=True)

    bounce_a = nc.dram_tensor("bounce_a", [PAD_N, HID], F32)
    bounce_b = nc.dram_tensor("bounce_b", [PAD_N, HID], F32)
    table = nc.dram_tensor("table", [TAB, HID], F32, addr_space="Shared")
    stat_i = nc.dram_tensor("stat_i", [1, 2 * HID], F32)
    stat_o = nc.dram_tensor("stat_o", [1, 2 * HID], F32, addr_space="Shared")

    ctx = contextlib.ExitStack()
    sb = lambda name, shp, dt=F32: ctx.enter_context(nc.sbuf_tensor(name, shp, dt))
    ps = lambda name, shp: ctx.enter_context(nc.psum_tensor(name, shp, F32))
    sem = lambda name: ctx.enter_context(nc.semaphore(name))

    with ctx:
        s_x = sb("s_x", [128, NCHUNK, INF])
        s_ga = sb("s_ga", [128, KA * 8], mybir.dt.int16)
        s_gb = sb("s_gb", [128, KB * 8], mybir.dt.int16)
        s_dg = sb("s_dg", [128, NCHUNK])
        s_id = sb("s_id", [128, 128])
        s_onc = sb("s_onc", [128, 1])
        s_onr = sb("s_onr", [1, 128])
        s_pmk = sb("s_pmk", [128, 1])
        s_w1 = sb("s_w1", [INF, HID])
        s_wsq = {nm: sb("s_" + nm, [HID, HID]) for nm in ["W2", "WX0", "WX1", "Wfc"]}
        s_vec = {nm: sb("sv_" + nm, [1, HID]) for nm in
                 ["b1", "b2", "bx0", "bx1", "bfc", "g1", "be1", "g2", "be2", "g3", "be3"]}
        s_rep = {nm: sb("sr_" + nm, [128, HID]) for nm in
                 ["b1", "b2", "bx0", "bx1", "bfc", "a", "beta"]}
        dinv = sb("dinv", [128, NCHUNK])
        dsq = sb("dsq", [128, NCHUNK])
        d2s = sb("d2s", [128, NCHUNK])
        tmpc = sb("tmpc", [128, NCHUNK])
        zz = sb("zz", [128, NCHUNK, HID])
        uu = sb("uu", [128, NCHUNK, HID])
        w0 = sb("w0", [128, NCHUNK, HID])
        hh = sb("hh", [128, NCHUNK, HID])
        bufA = [sb(f"bufA{i}", [128, GMAX, HID]) for i in range(2)]
        bufB = [sb(f"bufB{i}", [128, GMAX, HID]) for i in range(2)]
        vtmp = sb("vtmp", [128, HID])
        s_ht = sb("s_ht", [HID, 128])
        s_xt = sb("s_xt", [128, 128])
        sA = sb("sA", [128, HID])
        sB = sb("sB", [128, HID])
        s_st = sb("s_st", [1, 2 * HID])
        v1 = sb("v1", [1, HID])
        v2 = sb("v2", [1, HID])
        v3 = sb("v3", [1, HID])
        v4 = sb("v4", [1, HID])
        vmax = sb("vmax", [128, NCHUNK])
        sq = sb("sq", [128, NCHUNK, HID])
        pT = ps("pT", [128, 128])
        pM = ps("pM", [128, HID])
        pR = ps("pR", [128, HID])
        pS1 = ps("pS1", [1, HID])
        pS2 = ps("pS2", [1, HID])

        S = {k: sem("sem_" + k) for k in
             ["dma", "coll", "mm", "dve", "act", "dma2", "q0", "q1", "q2", "q3"]}
        C = {k: 0 for k in S}

        with nc.Block() as block:

            @block.gpsimd
            def _(g):
                V, T, A, Y = nc.vector, nc.tensor, nc.scalar, nc.sync

                def w(eng, *keys):
                    for k in keys:
                        eng.wait_ge(S[k], C[k])

                def dma(out, in_, eng=g):
                    eng.dma_start(out=out, in_=in_).then_inc(S["dma"], 16)
                    C["dma"] += 16

                def vop(fn, *a, **kw):
                    fn(*a, **kw).then_inc(S["dve"], 1)
                    C["dve"] += 1
                    V.wait_ge(S["dve"], C["dve"])

                def top(fn, *a, **kw):
                    fn(*a, **kw).then_inc(S["mm"], 1)
                    C["mm"] += 1
                    T.wait_ge(S["mm"], C["mm"])

                def aop(fn, *a, **kw):
                    fn(*a, **kw).then_inc(S["act"], 1)
                    C["act"] += 1
                    A.wait_ge(S["act"], C["act"])

                g.load_library(_mlp_lib)

                # ---- init loads ----
                dma(s_x[:, :, :], xs[:, :].rearrange("(c p) f -> p c f", p=128))
                dma(s_ga[:, :], gxa[:, :])
                dma(s_gb[:, :], gxb[:, :])
                dma(s_dg[:, :], dgp[:, :])
                dma(s_id[:, :], idq[:, :])
                dma(s_onc[:, :], onc[:, :])
                dma(s_onr[:, :], onr[:, :])
                dma(s_pmk[:, :], pmk[:, :])
                dma(s_w1[:, :], wts["W1"][:, :])
                for nm in s_wsq:
                    dma(s_wsq[nm][:, :], wts[nm][:, :])
                for nm in s_vec:
                    dma(s_vec[nm][:, :], vecs[nm][:, :])

                # degree-derived vectors
                w(A, "dma")
                aop(A.activation, dsq[:, :], s_dg[:, :], mybir.ActivationFunctionType.Sqrt)
                w(V, "act")
                vop(V.reciprocal, dinv[:, :], dsq[:, :])
                vop(V.tensor_mul, tmpc[:, :], dinv[:, :], dinv[:, :])
                vop(V.tensor_scalar_mul, d2s[:, :], tmpc[:, :], 1.0 - ALPHA)

                # replicate bias vectors across partitions: ones_row.T @ vec
                w(T, "dma")
                for nm in ["b1", "b2", "bx0", "bx1", "bfc"]:
                    top(T.matmul, pR[:, :], s_onr[:, :], s_vec[nm][:, :], start=True, stop=True)
                    w(V, "mm")
                    vop(V.tensor_copy, s_rep[nm][:, :], pR[:, :])
                    w(T, "dve")

                def replicate(vec_ap, dst_rep):
                    w(T, "dve", "act")
                    top(T.matmul, pR[:, :], s_onr[:, :], vec_ap, start=True, stop=True)
                    w(V, "mm")
                    vop(V.tensor_copy, dst_rep[:, :], pR[:, :])
                    w(T, "dve")

                # layer-1 table: uu = dinv * (x @ W1)
                w(T, "dve")
                for c in range(NCHUNK):
                    top(T.transpose, pT[:, :], s_x[:, c, :], s_id[:, :])
                    w(V, "mm")
                    vop(V.tensor_copy, s_xt[:, :], pT[:, :])
                    w(T, "dve")
                    top(T.matmul, pM[:, :], s_xt[:, :], s_w1[:, :], start=True, stop=True)
                    w(V, "mm")
                    vop(V.tensor_tensor, uu[:, c, :], pM[:, :],
                        _bcast_col(dinv[:, c:c + 1]), op=mybir.AluOpType.mult)
                    w(T, "dve")

                bb = {"cur": bounce_a, "nxt": bounce_b}
                qn = {"i": 0}

                def propagate(update=False, prebounced=False):
                    """AllGather uu -> table; dma_gather + segment-sum -> zz."""
                    bounce = bb["cur"]
                    if not prebounced:
                        vop(V.tensor_tensor, uu[:, 48, :], uu[:, 48, :],
                            _bcast_col(s_pmk[:, 0:1]), op=mybir.AluOpType.mult)
                        w(g, "dve")
                        dma(bounce[:, :].rearrange("(c p) f -> p c f", p=128), uu[:, :, :])
                        g.wait_ge(S["dma"], C["dma"])
                    else:
                        g.wait_ge(S["dma2"], C["dma2"])
                    g.collective_compute(
                        "AllGather", mybir.AluOpType.bypass,
                        replica_groups=[list(range(NCORES))],
                        ins=[bounce.ap().opt()], outs=[table.ap().opt()],
                    ).then_inc(S["coll"], 1)
                    C["coll"] += 1
                    g.wait_ge(S["coll"], C["coll"])
                    V.wait_ge(S["dma2"], C["dma2"])  # uu WAR vs sync bounces
                    red_done = []
                    gq = []
                    for gi, grp in enumerate(groups):
                        a0, a1 = aoff[grp[0]], aoff[grp[-1] + 1]
                        b0, b1 = boff[grp[0]], boff[grp[-1] + 1]
                        if gi >= 2:
                            g.wait_ge(S["dve"], red_done[gi - 2])
                        qa = "q%d" % (qn["i"] % 4)
                        qn["i"] += 1
                        qb = "q%d" % (qn["i"] % 4)
                        qn["i"] += 1
                        g.dma_gather(
                            out_ap=bufA[gi % 2][:, 0:a1 - a0, :],
                            in_ap=table[0:HALF, :],
                            idxs_ap=s_ga[:, 8 * a0:8 * a1],
                            num_idxs=128 * (a1 - a0),
                            num_idxs_reg=128 * (a1 - a0),
                            elem_size=HID,
                            single_packet=False,
                            queue_num=int(qa[1]),
                        ).then_inc(S[qa], 16)
                        C[qa] += 16
                        g.dma_gather(
                            out_ap=bufB[gi % 2][:, 0:b1 - b0, :],
                            in_ap=table[HALF:TAB, :],
                            idxs_ap=s_gb[:, 8 * b0:8 * b1],
                            num_idxs=128 * (b1 - b0),
                            num_idxs_reg=128 * (b1 - b0),
                            elem_size=HID,
                            single_packet=False,
                            queue_num=int(qb[1]),
                        ).then_inc(S[qb], 16)
                        C[qb] += 16
                        gq.append((qa, C[qa], qb, C[qb]))
                        qa_, na_, qb_, nb_ = gq[gi]
                        V.wait_ge(S[qa_], na_)
                        V.wait_ge(S[qb_], nb_)
                        for c in grp:
                            vop(V.tensor_reduce, zz[:, c, :],
                                _perm_kf(bufA[gi % 2], aoff[c] - a0, kA[c]),
                                mybir.AxisListType.X, mybir.AluOpType.add)
                            vop(V.tensor_reduce, vtmp[:, :],
                                _perm_kf(bufB[gi % 2], boff[c] - b0, kB[c]),
                                mybir.AxisListType.X, mybir.AluOpType.add)
                            vop(V.tensor_add, zz[:, c, :], zz[:, c, :], vtmp[:, :])
                            if update:
                                vop(V.tensor_tensor, uu[:, c, :], zz[:, c, :],
                                    _bcast_col(d2s[:, c:c + 1]), op=mybir.AluOpType.mult)
                                vop(V.tensor_add, uu[:, c, :], uu[:, c, :], w0[:, c, :])
                                Y.wait_ge(S["dve"], C["dve"])
                                Y.dma_start(out=bb["nxt"][128 * c:128 * (c + 1), :],
                                            in_=uu[:, c, :]).then_inc(S["dma2"], 16)
                                C["dma2"] += 16
                        red_done.append(C["dve"])
                    if update:
                        bb["cur"], bb["nxt"] = bb["nxt"], bb["cur"]

                def matmul_layer(w_sb, dst, scale_vec):
                    w(T, "dve", "act")
                    for c in range(NCHUNK):
                        top(T.transpose, pT[0:HID, :], hh[:, c, :], s_id[:, :])
                        w(V, "mm")
                        vop(V.tensor_copy, s_ht[:, :], pT[0:HID, :])
                        w(T, "dve")
                        top(T.matmul, pM[:, :], s_ht[:, :], w_sb[:, :], start=True, stop=True)
                        w(V, "mm")
                        vop(V.tensor_tensor, dst[:, c, :], pM[:, :],
                            _bcast_col(scale_vec[:, c:c + 1]), op=mybir.AluOpType.mult)
                        w(T, "dve")

                def bn_relu(bias_nm, g_nm, be_nm):
                    vop(V.tensor_tensor, zz[:, :, :], zz[:, :, :],
                        _bcast_f(dinv[:, :]), op=mybir.AluOpType.mult)
                    vop(V.tensor_tensor, zz[:, :, :], zz[:, :, :],
                        _bcast_rep(s_rep[bias_nm][:, :]), op=mybir.AluOpType.add)
                    b = zz[:, :, :]
                    zzkf = AP(b.tensor, b.offset, [b.ap[0], [1, HID], [HID, NCHUNK]])
                    vop(V.tensor_reduce, sA[:, :], zzkf,
                        mybir.AxisListType.X, mybir.AluOpType.add)
                    vop(V.tensor_mul, sq[:, :, :], zz[:, :, :], zz[:, :, :])
                    bq = sq[:, :, :]
                    zqkf = AP(bq.tensor, bq.offset, [bq.ap[0], [1, HID], [HID, NCHUNK]])
                    vop(V.tensor_reduce, sB[:, :], zqkf,
                        mybir.AxisListType.X, mybir.AluOpType.add)
                    w(T, "dve")
                    top(T.matmul, pS1[:, :], s_onc[:, :], sA[:, :], start=True, stop=True)
                    top(T.matmul, pS2[:, :], s_onc[:, :], sB[:, :], start=True, stop=True)
                    w(V, "mm")
                    vop(V.tensor_copy, s_st[0:1, 0:HID], pS1[:, :])
                    vop(V.tensor_copy, s_st[0:1, HID:2 * HID], pS2[:, :])
                    w(g, "dve")
                    dma(stat_i[:, :], s_st[:, :])
                    g.wait_ge(S["dma"], C["dma"])
                    g.collective_compute(
                        "AllReduce", mybir.AluOpType.add,
                        replica_groups=[list(range(NCORES))],
                        ins=[stat_i.ap().opt()], outs=[stat_o.ap().opt()],
                    ).then_inc(S["coll"], 1)
                    C["coll"] += 1
                    g.wait_ge(S["coll"], C["coll"])
                    dma(s_st[:, :], stat_o[:, :])
                    w(V, "dma")
                    npad = float(NCORES * (PAD_N - REAL))
                    vop(V.tensor_scalar_mul, v1[:, :], s_vec[bias_nm][:, :], npad)
                    vop(V.tensor_sub, v1[:, :], s_st[0:1, 0:HID], v1[:, :])
                    vop(V.tensor_scalar_mul, v1[:, :], v1[:, :], 1.0 / N)       # mean
                    vop(V.tensor_mul, v2[:, :], s_vec[bias_nm][:, :], s_vec[bias_nm][:, :])
                    vop(V.tensor_scalar_mul, v2[:, :], v2[:, :], npad)
                    vop(V.tensor_sub, v2[:, :], s_st[0:1, HID:2 * HID], v2[:, :])
                    vop(V.tensor_scalar_mul, v2[:, :], v2[:, :], 1.0 / N)       # E[x^2]
                    vop(V.tensor_mul, v3[:, :], v1[:, :], v1[:, :])
                    vop(V.tensor_sub, v2[:, :], v2[:, :], v3[:, :])             # var
                    vop(V.tensor_scalar_add, v2[:, :], v2[:, :], BN_EPS)
                    w(A, "dve")
                    aop(A.activation, v3[:, :], v2[:, :], mybir.ActivationFunctionType.Sqrt)
                    w(V, "act")
                    vop(V.reciprocal, v4[:, :], v3[:, :])                        # rstd
                    vop(V.tensor_mul, v4[:, :], v4[:, :], s_vec[g_nm][:, :])     # a
                    vop(V.tensor_mul, v3[:, :], v1[:, :], v4[:, :])
                    vop(V.tensor_sub, v3[:, :], s_vec[be_nm][:, :], v3[:, :])    # beta
                    replicate(v4[:, :], s_rep["a"])
                    replicate(v3[:, :], s_rep["beta"])
                    w(V, "dve")
                    vop(V.tensor_tensor, hh[:, :, :], zz[:, :, :],
                        _bcast_rep(s_rep["a"][:, :]), op=mybir.AluOpType.mult)
                    vop(V.tensor_tensor, hh[:, :, :], hh[:, :, :],
                        _bcast_rep(s_rep["beta"][:, :]), op=mybir.AluOpType.add)
                    vop(V.tensor_scalar_max, hh[:, :, :], hh[:, :, :], 0.0)

                # ---- 4 GCN layers ----
                layer_params = [("b1", "g1", "be1", "W2", "b2"),
                                ("b2", "g2", "be2", "WX0", "bx0"),
                                ("bx0", "g3", "be3", "WX1", "bx1"),
                                ("bx1", "g3", "be3", None, None)]
                for li, (bias_nm, g_nm, be_nm, next_w, _nb) in enumerate(layer_params):
                    propagate()
                    bn_relu(bias_nm, g_nm, be_nm)
                    if next_w is not None:
                        matmul_layer(s_wsq[next_w], uu, dinv)
                    else:
                        vop(V.tensor_tensor, uu[:, :, :], hh[:, :, :],
                            _bcast_f(dinv[:, :]), op=mybir.AluOpType.mult)
                        vop(V.tensor_scalar_mul, w0[:, :, :], uu[:, :, :], ALPHA)
                        vop(V.tensor_tensor, d2s[:, 48:49], d2s[:, 48:49],
                            s_pmk[:, 0:1], op=mybir.AluOpType.mult)
                        vop(V.tensor_tensor, w0[:, 48, :], w0[:, 48, :],
                            _bcast_col(s_pmk[:, 0:1]), op=mybir.AluOpType.mult)

                # ---- APPNP power iterations ----
                for _k in range(K_STEPS):
                    propagate(update=True, prebounced=(_k > 0))

                # ---- final: h = uu * sqrt(deg); out = log_softmax(h @ Wfc + bfc) ----
                vop(V.tensor_tensor, hh[:, :, :], uu[:, :, :],
                    _bcast_f(dsq[:, :]), op=mybir.AluOpType.mult)
                w(T, "dve", "act")
                for c in range(NCHUNK):
                    top(T.transpose, pT[0:HID, :], hh[:, c, :], s_id[:, :])
                    w(V, "mm")
                    vop(V.tensor_copy, s_ht[:, :], pT[0:HID, :])
                    w(T, "dve")
                    top(T.matmul, pM[:, :], s_ht[:, :], s_wsq["Wfc"][:, :], start=True, stop=True)
                    w(V, "mm")
                    vop(V.tensor_tensor, zz[:, c, :], pM[:, :],
                        s_rep["bfc"][:, :], op=mybir.AluOpType.add)
                    w(T, "dve")
                vop(V.tensor_reduce, vmax[:, :], zz[:, :, :],
                    mybir.AxisListType.X, mybir.AluOpType.max)
                vop(V.tensor_tensor, zz[:, :, :], zz[:, :, :],
                    _bcast_f(vmax[:, :]), op=mybir.AluOpType.subtract)
                w(A, "dve")
                aop(A.activation, sq[:, :, :], zz[:, :, :],
                    mybir.ActivationFunctionType.Exp)
                w(V, "act")
                vop(V.tensor_reduce, vmax[:, :], sq[:, :, :],
                    mybir.AxisListType.X, mybir.AluOpType.add)
                w(A, "dve")
                aop(A.activation, tmpc[:, :], vmax[:, :], mybir.ActivationFunctionType.Ln)
                w(V, "act")
                vop(V.tensor_tensor, zz[:, :, :], zz[:, :, :],
                    _bcast_f(tmpc[:, :]), op=mybir.AluOpType.subtract)
                w(g, "dve")
                dma(out_d[0:6144, :].rearrange("(c p) f -> p c f", p=128), zz[:, 0:48, :])
                dma(out_d[6144:REAL, :], zz[0:106, 48, :])
                g.wait_ge(S["dma"], C["dma"])

    nc.compile()
    return nc


def _in_maps(inputs, plan):
    x = np.asarray(inputs["x"], np.float32)
    order = plan["order"]
    Wx = np.asarray(inputs["Wx"], np.float32)
    bx = np.asarray(inputs["bx"], np.float32)
    common = {
        "ident": np.eye(128, dtype=np.float32),
        "onescol": np.ones((128, 1), np.float32),
        "onesrow": np.ones((1, 128), np.float32),
        "padmask": (np.arange(128) < REAL - 48 * 128).astype(np.float32)[:, None],
        "W1": np.asarray(inputs["W1"], np.float32),
        "W2": np.asarray(inputs["W2"], np.float32),
        "WX0": Wx[0], "WX1": Wx[1],
        "Wfc": np.asarray(inputs["Wfc"], np.float32),
        "b1": np.asarray(inputs["b1"], np.float32)[None, :],
        "b2": np.asarray(inputs["b2"], np.float32)[None, :],
        "bx0": bx[0][None, :], "bx1": bx[1][None, :],
        "bfc": np.asarray(inputs["bfc"], np.float32)[None, :],
        "g1": np.asarray(inputs["g1"], np.float32)[None, :],
        "be1": np.asarray(inputs["be1"], np.float32)[None, :],
        "g2": np.asarray(inputs["g2"], np.float32)[None, :],
        "be2": np.asarray(inputs["be2"], np.float32)[None, :],
        "g3": np.asarray(inputs["g3"], np.float32)[None, :],
        "be3": np.asarray(inputs["be3"], np.float32)[None, :],
    }
    core_of, lr_of = plan["core_of"], plan["lr_of"]
    maps = []
    for c in range(NCORES):
        xs = np.zeros((PAD_N, INF), np.float32)
        mine = np.where(core_of == c)[0]
        xs[lr_of[mine]] = x[mine]
        m = {"xs": xs, "gxa": plan["gAw"][c], "gxb": plan["gBw"][c],
             "degp": plan["deg_pc"][c]}
        m.update(common)
        maps.append(m)
    return maps


def _unpermute(results, plan):
    core_of, lr_of = plan["core_of"], plan["lr_of"]
    out = np.empty((N, NCLS), np.float32)
    for c in range(NCORES):
        mine = np.where(core_of == c)[0]
        out[mine] = results[c]["out"][lr_of[mine]]
    return out


def kernel(**inputs):
    plan = _plan(np.asarray(inputs["edge_index"]))
    nc = _build(plan)
    maps = _in_maps(inputs, plan)
    res = run_bass_kernel_spmd(nc, maps, core_ids=list(range(NCORES)), trace=False)
    return _unpermute(res.results, plan)


# revision 6
# speedup vs baseline: 1.2602x; 1.2418x over previous
"""APPNP+BN GNN kernel for 8 Trainium2 NeuronCores (Bass).

Sharding: nodes are degree-sorted and dealt round-robin to 8 cores (node/data
parallel).  Each core owns 6250 destination rows (padded to 6272 = 49 chunks
of 128).  Per propagate step: every core's pre-scaled activation shard is
AllGathered into a full node table in DRAM; each core fetches its edges'
source rows with dma_gather (4 SWDGE queues, int16 indices, so the table is
addressed as two <32768-row halves; a host-side greedy pass balances each
destination row's sources across the halves), segment-sums them on the Vector
engine (fixed slot grid per half: chunk c has kA_c/kB_c slot columns = max
per-half degree in chunk), and applies the D^-1/2 scalings, biases, BN (batch
stats via ones-matmul partition reduction + tiny AllReduce), ReLU and weight
matmuls (TensorE, via identity transposes).  Output is log_softmax rows which
the host inverse-permutes.
"""

import contextlib
import numpy as np

import concourse.bacc as bacc
import concourse.mybir as mybir
from concourse.bass import AP
from concourse.bass_utils import run_bass_kernel_spmd
from concourse.library_config import mlp as _mlp_lib

N = 50000
E = 800000
INF = 128
HID = 64
NCLS = 64
NCORES = 8
PAD_N = 6272          # 49 * 128 rows per core
NCHUNK = 49
REAL = 6250
TAB = PAD_N * NCORES  # 50176 table rows
HALF = TAB // 2       # 25088: gather half A = rows [0, HALF), B = [HALF, TAB)
ZROW = 6250           # zero pad row, same local index in both halves
ALPHA = 0.1
K_STEPS = 10
BN_EPS = 1e-5
GMAX = 32             # max slot columns per dma_gather op
NBUF = 4              # gather buffer pipeline depth
F32 = mybir.dt.float32


def _balance_bits(src, dst, ranks, deg):
    """Greedy per-node half-assignment: nodes with bit 0 land in table half A
    (cores 0-3), bit 1 in half B.  Exactly 512 of each per 1024-rank span so
    the round-robin core deal stays balanced.  Objective: for every dest node,
    split its in-edges' sources evenly between halves (keeps per-chunk max
    slot counts kA+kB close to kc)."""
    nspan = (N + 1023) // 1024
    capA = np.zeros(nspan, np.int64)
    capB = np.zeros(nspan, np.int64)
    for s in range(nspan):
        tot = min(1024, N - s * 1024)
        capA[s] = tot // 2
        capB[s] = tot - tot // 2
    # out-adjacency: for node n (as source), list of dests
    order_e = np.argsort(src, kind="stable")
    s_sorted = src[order_e]
    d_sorted = dst[order_e]
    starts = np.searchsorted(s_sorted, np.arange(N))
    ends = np.searchsorted(s_sorted, np.arange(N) + 1)
    cntA = np.zeros(N, np.int32)
    cntB = np.zeros(N, np.int32)
    bit = np.zeros(N, np.int8)
    # process in descending out-degree
    for n in np.argsort(-(ends - starts), kind="stable"):
        sp = ranks[n] // 1024
        ds = d_sorted[starts[n]:ends[n]]
        if capA[sp] == 0:
            b = 1
        elif capB[sp] == 0:
            b = 0
        else:
            # penalize pushing any dest row past ceil(deg/2) on either half
            halfcap = (deg[ds] + 1) // 2
            pa = int(np.sum(np.maximum(cntA[ds] + 1 - halfcap, 0) ** 2) - np.sum(np.maximum(cntA[ds] - halfcap, 0) ** 2))
            pb = int(np.sum(np.maximum(cntB[ds] + 1 - halfcap, 0) ** 2) - np.sum(np.maximum(cntB[ds] - halfcap, 0) ** 2))
            if pa != pb:
                b = 0 if pa < pb else 1
            else:
                b = 0 if int(cntA[ds].sum()) <= int(cntB[ds].sum()) else 1
        bit[n] = b
        if b == 0:
            capA[sp] -= 1
            cntA[ds] += 1
        else:
            capB[sp] -= 1
            cntB[ds] += 1
    return bit


def _plan(edge_index):
    src = np.asarray(edge_index[0], dtype=np.int64)
    dst = np.asarray(edge_index[1], dtype=np.int64)
    loop = np.arange(N, dtype=np.int64)
    src = np.concatenate([src, loop])
    dst = np.concatenate([dst, loop])
    deg = np.bincount(dst, minlength=N)  # >= 1 everywhere (self loops)

    order = np.argsort(-deg, kind="stable")          # rank -> node
    ranks = np.empty(N, np.int64)
    ranks[order] = np.arange(N)

    bit = _balance_bits(src, dst, ranks, deg)
    # within each 1024-rank span: bit-0 nodes -> cores 0-3, bit-1 -> cores 4-7
    core_of = np.empty(N, np.int64)
    lr_of = np.empty(N, np.int64)
    for s in range((N + 1023) // 1024):
        span = order[s * 1024:(s + 1) * 1024]        # nodes in rank order
        a = span[bit[span] == 0]
        b = span[bit[span] == 1]
        for g, grp in ((0, a), (4, b)):
            for i, n in enumerate(grp):
                core_of[n] = g + i % 4
                lr_of[n] = s * 128 + i // 4
    pi = core_of * PAD_N + lr_of                      # node -> table row

    pc_deg = np.ones((NCORES, PAD_N), np.float32)
    pc_deg[core_of, lr_of] = deg

    s_row = pi[src]
    d_core = core_of[dst]
    d_lr = lr_of[dst]
    half_of = (s_row >= HALF).astype(np.int64)

    # per-chunk per-half max slot counts (shared across cores: SPMD)
    kA = np.zeros(NCHUNK, np.int64)
    kB = np.zeros(NCHUNK, np.int64)
    cnt = np.zeros((2, NCORES, PAD_N), np.int64)
    np.add.at(cnt, (half_of, d_core, d_lr), 1)
    for ch in range(NCHUNK):
        kA[ch] = max(1, int(cnt[0, :, ch * 128:(ch + 1) * 128].max()))
        kB[ch] = max(1, int(cnt[1, :, ch * 128:(ch + 1) * 128].max()))
    KA = int(kA.sum())
    KB = int(kB.sum())
    aoff = np.concatenate([[0], np.cumsum(kA)]).astype(np.int64)
    boff = np.concatenate([[0], np.cumsum(kB)]).astype(np.int64)

    # chunk groups: consecutive chunks with sum kA <= GMAX and sum kB <= GMAX
    groups = []
    cur = [0]
    sa = int(kA[0])
    sb = int(kB[0])
    for ch in range(1, NCHUNK):
        if sa + kA[ch] <= GMAX and sb + kB[ch] <= GMAX:
            cur.append(ch)
            sa += int(kA[ch])
            sb += int(kB[ch])
        else:
            groups.append(cur)
            cur = [ch]
            sa = int(kA[ch])
            sb = int(kB[ch])
    groups.append(cur)

    # per-core index grids [128, K] (int16 local rows, pad = ZROW)
    gA = np.full((NCORES, 128, KA), ZROW, np.int16)
    gB = np.full((NCORES, 128, KB), ZROW, np.int16)
    for c in range(NCORES):
        for h, (gg, off) in enumerate(((gA, aoff), (gB, boff))):
            m = (d_core == c) & (half_of == h)
            ls = d_lr[m]
            sr = s_row[m] - h * HALF
            o = np.argsort(ls, kind="stable")
            ls = ls[o]
            sr = sr[o]
            bc = np.bincount(ls, minlength=PAD_N)
            eoff = np.concatenate([[0], np.cumsum(bc)])
            t = np.arange(ls.size) - eoff[ls]
            p = ls % 128
            ch = ls // 128
            col = off[ch] + t
            gg[c, p, col] = sr.astype(np.int16)

    # wrapped int16 index streams for dma_gather: index i = col*128 + p lives
    # at [i % 16 + 16*rep, i // 16]
    def wrap(gg, K):
        out = np.empty((NCORES, 128, K * 8), np.int16)
        for c in range(NCORES):
            flat = gg[c].T.reshape(-1)           # i = col*128 + p
            w = flat.reshape(-1, 16).T           # [16, K*8]
            out[c] = np.tile(w, (8, 1))
        return out

    deg_pc = np.ones((NCORES, 128, NCHUNK), np.float32)
    for c in range(NCORES):
        deg_pc[c] = pc_deg[c].reshape(NCHUNK, 128).T

    return dict(order=order, kA=kA, kB=kB, KA=KA, KB=KB, aoff=aoff, boff=boff,
                groups=groups, gAw=wrap(gA, KA), gBw=wrap(gB, KB),
                deg_pc=deg_pc, core_of=core_of, lr_of=lr_of)


def _bcast_f(vec2d, nf=HID):
    b = vec2d
    return AP(b.tensor, b.offset, list(b.ap) + [[0, nf]])


def _bcast_col(vec2d_col, nf=HID):
    b = vec2d_col
    return AP(b.tensor, b.offset, [b.ap[0], [0, nf]])


def _bcast_rep(rep, nch=NCHUNK):
    b = rep
    return AP(b.tensor, b.offset, [b.ap[0], [0, nch], b.ap[1]])


def _perm_kf(buf, c0, k):
    """buf[:, c0:c0+k, :] ([128, k, 64]) -> AP ordered [p, f, k] so the
    innermost (reduced) axis is the slot axis."""
    b = buf[:, c0:c0 + k, :]
    return AP(b.tensor, b.offset, [b.ap[0], [1, HID], [HID, k]])


def _build(plan):
    kA = [int(x) for x in plan["kA"]]
    kB = [int(x) for x in plan["kB"]]
    aoff = [int(x) for x in plan["aoff"]]
    boff = [int(x) for x in plan["boff"]]
    groups = plan["groups"]
    KA, KB = plan["KA"], plan["KB"]
    nc = bacc.Bacc(target_bir_lowering=False, num_swdge_queues=4)

    xs = nc.declare_dram_parameter("xs", [PAD_N, INF], F32, isOutput=False)
    gxa = nc.declare_dram_parameter("gxa", [128, KA * 8], mybir.dt.int16, isOutput=False)
    gxb = nc.declare_dram_parameter("gxb", [128, KB * 8], mybir.dt.int16, isOutput=False)
    dgp = nc.declare_dram_parameter("degp", [128, NCHUNK], F32, isOutput=False)
    idq = nc.declare_dram_parameter("ident", [128, 128], F32, isOutput=False)
    onc = nc.declare_dram_parameter("onescol", [128, 1], F32, isOutput=False)
    onr = nc.declare_dram_parameter("onesrow", [1, 128], F32, isOutput=False)
    pmk = nc.declare_dram_parameter("padmask", [128, 1], F32, isOutput=False)
    wts = {}
    for nm, shp in [("W1", [INF, HID]), ("W2", [HID, HID]), ("WX0", [HID, HID]),
                    ("WX1", [HID, HID]), ("Wfc", [HID, NCLS])]:
        wts[nm] = nc.declare_dram_parameter(nm, shp, F32, isOutput=False)
    vecs = {}
    for nm in ["b1", "b2", "bx0", "bx1", "bfc", "g1", "be1", "g2", "be2", "g3", "be3"]:
        vecs[nm] = nc.declare_dram_parameter(nm, [1, HID], F32, isOutput=False)
    out_d = nc.declare_dram_parameter("out", [REAL, NCLS], F32, isOutput=True)

    bounce_a = nc.dram_tensor("bounce_a", [PAD_N, HID], F32)
    bounce_b = nc.dram_tensor("bounce_b", [PAD_N, HID], F32)
    table = nc.dram_tensor("table", [TAB, HID], F32, addr_space="Shared")
    stat_i = nc.dram_tensor("stat_i", [1, 2 * HID], F32)
    stat_o = nc.dram_tensor("stat_o", [1, 2 * HID], F32, addr_space="Shared")

    ctx = contextlib.ExitStack()
    sb = lambda name, shp, dt=F32: ctx.enter_context(nc.sbuf_tensor(name, shp, dt))
    ps = lambda name, shp: ctx.enter_context(nc.psum_tensor(name, shp, F32))
    sem = lambda name: ctx.enter_context(nc.semaphore(name))

    with ctx:
        s_x = sb("s_x", [128, NCHUNK, INF])
        s_ga = sb("s_ga", [128, KA * 8], mybir.dt.int16)
        s_gb = sb("s_gb", [128, KB * 8], mybir.dt.int16)
        s_dg = sb("s_dg", [128, NCHUNK])
        s_id = sb("s_id", [128, 128])
        s_onc = sb("s_onc", [128, 1])
        s_onr = sb("s_onr", [1, 128])
        s_pmk = sb("s_pmk", [128, 1])
        s_w1 = sb("s_w1", [INF, HID])
        s_wsq = {nm: sb("s_" + nm, [HID, HID]) for nm in ["W2", "WX0", "WX1", "Wfc"]}
        s_vec = {nm: sb("sv_" + nm, [1, HID]) for nm in
                 ["b1", "b2", "bx0", "bx1", "bfc", "g1", "be1", "g2", "be2", "g3", "be3"]}
        s_rep = {nm: sb("sr_" + nm, [128, HID]) for nm in
                 ["b1", "b2", "bx0", "bx1", "bfc", "a", "beta"]}
        dinv = sb("dinv", [128, NCHUNK])
        dsq = sb("dsq", [128, NCHUNK])
        d2s = sb("d2s", [128, NCHUNK])
        tmpc = sb("tmpc", [128, NCHUNK])
        zz = sb("zz", [128, NCHUNK, HID])
        uu = sb("uu", [128, NCHUNK, HID])
        w0 = sb("w0", [128, NCHUNK, HID])
        hh = sb("hh", [128, NCHUNK, HID])
        bufA = [sb(f"bufA{i}", [128, GMAX, HID]) for i in range(NBUF)]
        bufB = [sb(f"bufB{i}", [128, GMAX, HID]) for i in range(NBUF)]
        vtmp = sb("vtmp", [128, HID])
        s_ht = sb("s_ht", [HID, 128])
        s_xt = sb("s_xt", [128, 128])
        sA = sb("sA", [128, HID])
        sB = sb("sB", [128, HID])
        s_st = sb("s_st", [1, 2 * HID])
        v1 = sb("v1", [1, HID])
        v2 = sb("v2", [1, HID])
        v3 = sb("v3", [1, HID])
        v4 = sb("v4", [1, HID])
        vmax = sb("vmax", [128, NCHUNK])
        sq = sb("sq", [128, NCHUNK, HID])
        pT = ps("pT", [128, 128])
        pM = ps("pM", [128, HID])
        pR = ps("pR", [128, HID])
        pS1 = ps("pS1", [1, HID])
        pS2 = ps("pS2", [1, HID])

        S = {k: sem("sem_" + k) for k in
             ["dma", "coll", "mm", "dve", "act", "dma2", "q0", "q1", "q2", "q3"]}
        C = {k: 0 for k in S}

        with nc.Block() as block:

            @block.gpsimd
            def _(g):
                V, T, A, Y = nc.vector, nc.tensor, nc.scalar, nc.sync

                def w(eng, *keys):
                    for k in keys:
                        eng.wait_ge(S[k], C[k])

                def dma(out, in_, eng=g):
                    eng.dma_start(out=out, in_=in_).then_inc(S["dma"], 16)
                    C["dma"] += 16

                def vop(fn, *a, **kw):
                    fn(*a, **kw).then_inc(S["dve"], 1)
                    C["dve"] += 1
                    V.wait_ge(S["dve"], C["dve"])

                def top(fn, *a, **kw):
                    fn(*a, **kw).then_inc(S["mm"], 1)
                    C["mm"] += 1
                    T.wait_ge(S["mm"], C["mm"])

                def aop(fn, *a, **kw):
                    fn(*a, **kw).then_inc(S["act"], 1)
                    C["act"] += 1
                    A.wait_ge(S["act"], C["act"])

                g.load_library(_mlp_lib)

                # ---- init loads ----
                dma(s_x[:, :, :], xs[:, :].rearrange("(c p) f -> p c f", p=128))
                dma(s_ga[:, :], gxa[:, :])
                dma(s_gb[:, :], gxb[:, :])
                dma(s_dg[:, :], dgp[:, :])
                dma(s_id[:, :], idq[:, :])
                dma(s_onc[:, :], onc[:, :])
                dma(s_onr[:, :], onr[:, :])
                dma(s_pmk[:, :], pmk[:, :])
                dma(s_w1[:, :], wts["W1"][:, :])
                for nm in s_wsq:
                    dma(s_wsq[nm][:, :], wts[nm][:, :])
                for nm in s_vec:
                    dma(s_vec[nm][:, :], vecs[nm][:, :])

                # degree-derived vectors
                w(A, "dma")
                aop(A.activation, dsq[:, :], s_dg[:, :], mybir.ActivationFunctionType.Sqrt)
                w(V, "act")
                vop(V.reciprocal, dinv[:, :], dsq[:, :])
                vop(V.tensor_mul, tmpc[:, :], dinv[:, :], dinv[:, :])
                vop(V.tensor_scalar_mul, d2s[:, :], tmpc[:, :], 1.0 - ALPHA)

                # replicate bias vectors across partitions: ones_row.T @ vec
                w(T, "dma")
                for nm in ["b1", "b2", "bx0", "bx1", "bfc"]:
                    top(T.matmul, pR[:, :], s_onr[:, :], s_vec[nm][:, :], start=True, stop=True)
                    w(V, "mm")
                    vop(V.tensor_copy, s_rep[nm][:, :], pR[:, :])
                    w(T, "dve")

                def replicate(vec_ap, dst_rep):
                    w(T, "dve", "act")
                    top(T.matmul, pR[:, :], s_onr[:, :], vec_ap, start=True, stop=True)
                    w(V, "mm")
                    vop(V.tensor_copy, dst_rep[:, :], pR[:, :])
                    w(T, "dve")

                # layer-1 table: uu = dinv * (x @ W1)
                w(T, "dve")
                for c in range(NCHUNK):
                    top(T.transpose, pT[:, :], s_x[:, c, :], s_id[:, :])
                    w(V, "mm")
                    vop(V.tensor_copy, s_xt[:, :], pT[:, :])
                    w(T, "dve")
                    top(T.matmul, pM[:, :], s_xt[:, :], s_w1[:, :], start=True, stop=True)
                    w(V, "mm")
                    vop(V.tensor_tensor, uu[:, c, :], pM[:, :],
                        _bcast_col(dinv[:, c:c + 1]), op=mybir.AluOpType.mult)
                    w(T, "dve")

                bb = {"cur": bounce_a, "nxt": bounce_b}
                qn = {"i": 0}

                def propagate(update=False, prebounced=False):
                    """AllGather uu -> table; dma_gather + segment-sum -> zz."""
                    bounce = bb["cur"]
                    if not prebounced:
                        vop(V.tensor_tensor, uu[:, 48, :], uu[:, 48, :],
                            _bcast_col(s_pmk[:, 0:1]), op=mybir.AluOpType.mult)
                        w(g, "dve")
                        dma(bounce[:, :].rearrange("(c p) f -> p c f", p=128), uu[:, :, :])
                        g.wait_ge(S["dma"], C["dma"])
                    else:
                        g.wait_ge(S["dma2"], C["dma2"])
                    g.collective_compute(
                        "AllGather", mybir.AluOpType.bypass,
                        replica_groups=[list(range(NCORES))],
                        ins=[bounce.ap().opt()], outs=[table.ap().opt()],
                    ).then_inc(S["coll"], 1)
                    C["coll"] += 1
                    g.wait_ge(S["coll"], C["coll"])
                    V.wait_ge(S["dma2"], C["dma2"])  # uu WAR vs sync bounces
                    red_done = []
                    gq = []
                    for gi, grp in enumerate(groups):
                        a0, a1 = aoff[grp[0]], aoff[grp[-1] + 1]
                        b0, b1 = boff[grp[0]], boff[grp[-1] + 1]
                        if gi >= NBUF:
                            g.wait_ge(S["dve"], red_done[gi - NBUF])
                        qa = "q%d" % (qn["i"] % 4)
                        qn["i"] += 1
                        qb = "q%d" % (qn["i"] % 4)
                        qn["i"] += 1
                        g.dma_gather(
                            out_ap=bufA[gi % NBUF][:, 0:a1 - a0, :],
                            in_ap=table[0:HALF, :],
                            idxs_ap=s_ga[:, 8 * a0:8 * a1],
                            num_idxs=128 * (a1 - a0),
                            num_idxs_reg=128 * (a1 - a0),
                            elem_size=HID,
                            single_packet=False,
                            queue_num=int(qa[1]),
                        ).then_inc(S[qa], 16)
                        C[qa] += 16
                        g.dma_gather(
                            out_ap=bufB[gi % NBUF][:, 0:b1 - b0, :],
                            in_ap=table[HALF:TAB, :],
                            idxs_ap=s_gb[:, 8 * b0:8 * b1],
                            num_idxs=128 * (b1 - b0),
                            num_idxs_reg=128 * (b1 - b0),
                            elem_size=HID,
                            single_packet=False,
                            queue_num=int(qb[1]),
                        ).then_inc(S[qb], 16)
                        C[qb] += 16
                        gq.append((qa, C[qa], qb, C[qb]))
                        qa_, na_, qb_, nb_ = gq[gi]
                        V.wait_ge(S[qa_], na_)
                        V.wait_ge(S[qb_], nb_)
                        for c in grp:
                            vop(V.tensor_reduce, zz[:, c, :],
                                _perm_kf(bufA[gi % NBUF], aoff[c] - a0, kA[c]),
                                mybir.AxisListType.X, mybir.AluOpType.add)
                            vop(V.tensor_reduce, vtmp[:, :],
                                _perm_kf(bufB[gi % NBUF], boff[c] - b0, kB[c]),
                                mybir.AxisListType.X, mybir.AluOpType.add)
                            vop(V.tensor_add, zz[:, c, :], zz[:, c, :], vtmp[:, :])
                            if update:
                                vop(V.tensor_tensor, uu[:, c, :], zz[:, c, :],
                                    _bcast_col(d2s[:, c:c + 1]), op=mybir.AluOpType.mult)
                                vop(V.tensor_add, uu[:, c, :], uu[:, c, :], w0[:, c, :])
                                Y.wait_ge(S["dve"], C["dve"])
                                Y.dma_start(out=bb["nxt"][128 * c:128 * (c + 1), :],
                                            in_=uu[:, c, :]).then_inc(S["dma2"], 16)
                                C["dma2"] += 16
                        red_done.append(C["dve"])
                    if update:
                        bb["cur"], bb["nxt"] = bb["nxt"], bb["cur"]

                def matmul_layer(w_sb, dst, scale_vec):
                    w(T, "dve", "act")
                    for c in range(NCHUNK):
                        top(T.transpose, pT[0:HID, :], hh[:, c, :], s_id[:, :])
                        w(V, "mm")
                        vop(V.tensor_copy, s_ht[:, :], pT[0:HID, :])
                        w(T, "dve")
                        top(T.matmul, pM[:, :], s_ht[:, :], w_sb[:, :], start=True, stop=True)
                        w(V, "mm")
                        vop(V.tensor_tensor, dst[:, c, :], pM[:, :],
                            _bcast_col(scale_vec[:, c:c + 1]), op=mybir.AluOpType.mult)
                        w(T, "dve")

                def bn_relu(bias_nm, g_nm, be_nm):
                    vop(V.tensor_tensor, zz[:, :, :], zz[:, :, :],
                        _bcast_f(dinv[:, :]), op=mybir.AluOpType.mult)
                    vop(V.tensor_tensor, zz[:, :, :], zz[:, :, :],
                        _bcast_rep(s_rep[bias_nm][:, :]), op=mybir.AluOpType.add)
                    b = zz[:, :, :]
                    zzkf = AP(b.tensor, b.offset, [b.ap[0], [1, HID], [HID, NCHUNK]])
                    vop(V.tensor_reduce, sA[:, :], zzkf,
                        mybir.AxisListType.X, mybir.AluOpType.add)
                    vop(V.tensor_mul, sq[:, :, :], zz[:, :, :], zz[:, :, :])
                    bq = sq[:, :, :]
                    zqkf = AP(bq.tensor, bq.offset, [bq.ap[0], [1, HID], [HID, NCHUNK]])
                    vop(V.tensor_reduce, sB[:, :], zqkf,
                        mybir.AxisListType.X, mybir.AluOpType.add)
                    w(T, "dve")
                    top(T.matmul, pS1[:, :], s_onc[:, :], sA[:, :], start=True, stop=True)
                    top(T.matmul, pS2[:, :], s_onc[:, :], sB[:, :], start=True, stop=True)
                    w(V, "mm")
                    vop(V.tensor_copy, s_st[0:1, 0:HID], pS1[:, :])
                    vop(V.tensor_copy, s_st[0:1, HID:2 * HID], pS2[:, :])
                    w(g, "dve")
                    dma(stat_i[:, :], s_st[:, :])
                    g.wait_ge(S["dma"], C["dma"])
                    g.collective_compute(
                        "AllReduce", mybir.AluOpType.add,
                        replica_groups=[list(range(NCORES))],
                        ins=[stat_i.ap().opt()], outs=[stat_o.ap().opt()],
                    ).then_inc(S["coll"], 1)
                    C["coll"] += 1
                    g.wait_ge(S["coll"], C["coll"])
                    dma(s_st[:, :], stat_o[:, :])
                    w(V, "dma")
                    npad = float(NCORES * (PAD_N - REAL))
                    vop(V.tensor_scalar_mul, v1[:, :], s_vec[bias_nm][:, :], npad)
                    vop(V.tensor_sub, v1[:, :], s_st[0:1, 0:HID], v1[:, :])
                    vop(V.tensor_scalar_mul, v1[:, :], v1[:, :], 1.0 / N)       # mean
                    vop(V.tensor_mul, v2[:, :], s_vec[bias_nm][:, :], s_vec[bias_nm][:, :])
                    vop(V.tensor_scalar_mul, v2[:, :], v2[:, :], npad)
                    vop(V.tensor_sub, v2[:, :], s_st[0:1, HID:2 * HID], v2[:, :])
                    vop(V.tensor_scalar_mul, v2[:, :], v2[:, :], 1.0 / N)       # E[x^2]
                    vop(V.tensor_mul, v3[:, :], v1[:, :], v1[:, :])
                    vop(V.tensor_sub, v2[:, :], v2[:, :], v3[:, :])             # var
                    vop(V.tensor_scalar_add, v2[:, :], v2[:, :], BN_EPS)
                    w(A, "dve")
                    aop(A.activation, v3[:, :], v2[:, :], mybir.ActivationFunctionType.Sqrt)
                    w(V, "act")
                    vop(V.reciprocal, v4[:, :], v3[:, :])                        # rstd
                    vop(V.tensor_mul, v4[:, :], v4[:, :], s_vec[g_nm][:, :])     # a
                    vop(V.tensor_mul, v3[:, :], v1[:, :], v4[:, :])
                    vop(V.tensor_sub, v3[:, :], s_vec[be_nm][:, :], v3[:, :])    # beta
                    replicate(v4[:, :], s_rep["a"])
                    replicate(v3[:, :], s_rep["beta"])
                    w(V, "dve")
                    vop(V.tensor_tensor, hh[:, :, :], zz[:, :, :],
                        _bcast_rep(s_rep["a"][:, :]), op=mybir.AluOpType.mult)
                    vop(V.tensor_tensor, hh[:, :, :], hh[:, :, :],
                        _bcast_rep(s_rep["beta"][:, :]), op=mybir.AluOpType.add)
                    vop(V.tensor_scalar_max, hh[:, :, :], hh[:, :, :], 0.0)

                # ---- 4 GCN layers ----
                layer_params = [("b1", "g1", "be1", "W2", "b2"),
                                ("b2", "g2", "be2", "WX0", "bx0"),
                                ("bx0", "g3", "be3", "WX1", "bx1"),
                                ("bx1", "g3", "be3", None, None)]
                for li, (bias_nm, g_nm, be_nm, next_w, _nb) in enumerate(layer_params):
                    propagate()
                    bn_relu(bias_nm, g_nm, be_nm)
                    if next_w is not None:
                        matmul_layer(s_wsq[next_w], uu, dinv)
                    else:
                        vop(V.tensor_tensor, uu[:, :, :], hh[:, :, :],
                            _bcast_f(dinv[:, :]), op=mybir.AluOpType.mult)
                        vop(V.tensor_scalar_mul, w0[:, :, :], uu[:, :, :], ALPHA)
                        vop(V.tensor_tensor, d2s[:, 48:49], d2s[:, 48:49],
                            s_pmk[:, 0:1], op=mybir.AluOpType.mult)
                        vop(V.tensor_tensor, w0[:, 48, :], w0[:, 48, :],
                            _bcast_col(s_pmk[:, 0:1]), op=mybir.AluOpType.mult)

                # ---- APPNP power iterations ----
                for _k in range(K_STEPS):
                    propagate(update=True, prebounced=(_k > 0))

                # ---- final: h = uu * sqrt(deg); out = log_softmax(h @ Wfc + bfc) ----
                vop(V.tensor_tensor, hh[:, :, :], uu[:, :, :],
                    _bcast_f(dsq[:, :]), op=mybir.AluOpType.mult)
                w(T, "dve", "act")
                for c in range(NCHUNK):
                    top(T.transpose, pT[0:HID, :], hh[:, c, :], s_id[:, :])
                    w(V, "mm")
                    vop(V.tensor_copy, s_ht[:, :], pT[0:HID, :])
                    w(T, "dve")
                    top(T.matmul, pM[:, :], s_ht[:, :], s_wsq["Wfc"][:, :], start=True, stop=True)
                    w(V, "mm")
                    vop(V.tensor_tensor, zz[:, c, :], pM[:, :],
                        s_rep["bfc"][:, :], op=mybir.AluOpType.add)
                    w(T, "dve")
                vop(V.tensor_reduce, vmax[:, :], zz[:, :, :],
                    mybir.AxisListType.X, mybir.AluOpType.max)
                vop(V.tensor_tensor, zz[:, :, :], zz[:, :, :],
                    _bcast_f(vmax[:, :]), op=mybir.AluOpType.subtract)
                w(A, "dve")
                aop(A.activation, sq[:, :, :], zz[:, :, :],
                    mybir.ActivationFunctionType.Exp)
                w(V, "act")
                vop(V.tensor_reduce, vmax[:, :], sq[:, :, :],
                    mybir.AxisListType.X, mybir.AluOpType.add)
                w(A, "dve")
                aop(A.activation, tmpc[:, :], vmax[:, :], mybir.ActivationFunctionType.Ln)
                w(V, "act")
                vop(V.tensor_tensor, zz[:, :, :], zz[:, :, :],
                    _bcast_f(tmpc[:, :]), op=mybir.AluOpType.subtract)
                w(g, "dve")
                dma(out_d[0:6144, :].rearrange("(c p) f -> p c f", p=128), zz[:, 0:48, :])
                dma(out_d[6144:REAL, :], zz[0:106, 48, :])
                g.wait_ge(S["dma"], C["dma"])

    nc.compile()
    return nc


def _in_maps(inputs, plan):
    x = np.asarray(inputs["x"], np.float32)
    order = plan["order"]
    Wx = np.asarray(inputs["Wx"], np.float32)
    bx = np.asarray(inputs["bx"], np.float32)
    common = {
        "ident": np.eye(128, dtype=np.float32),
        "onescol": np.ones((128, 1), np.float32),
        "onesrow": np.ones((1, 128), np.float32),
        "padmask": (np.arange(128) < REAL - 48 * 128).astype(np.float32)[:, None],
        "W1": np.asarray(inputs["W1"], np.float32),
        "W2": np.asarray(inputs["W2"], np.float32),
        "WX0": Wx[0], "WX1": Wx[1],
        "Wfc": np.asarray(inputs["Wfc"], np.float32),
        "b1": np.asarray(inputs["b1"], np.float32)[None, :],
        "b2": np.asarray(inputs["b2"], np.float32)[None, :],
        "bx0": bx[0][None, :], "bx1": bx[1][None, :],
        "bfc": np.asarray(inputs["bfc"], np.float32)[None, :],
        "g1": np.asarray(inputs["g1"], np.float32)[None, :],
        "be1": np.asarray(inputs["be1"], np.float32)[None, :],
        "g2": np.asarray(inputs["g2"], np.float32)[None, :],
        "be2": np.asarray(inputs["be2"], np.float32)[None, :],
        "g3": np.asarray(inputs["g3"], np.float32)[None, :],
        "be3": np.asarray(inputs["be3"], np.float32)[None, :],
    }
    core_of, lr_of = plan["core_of"], plan["lr_of"]
    maps = []
    for c in range(NCORES):
        xs = np.zeros((PAD_N, INF), np.float32)
        mine = np.where(core_of == c)[0]
        xs[lr_of[mine]] = x[mine]
        m = {"xs": xs, "gxa": plan["gAw"][c], "gxb": plan["gBw"][c],
             "degp": plan["deg_pc"][c]}
        m.update(common)
        maps.append(m)
    return maps


def _unpermute(results, plan):
    core_of, lr_of = plan["core_of"], plan["lr_of"]
    out = np.empty((N, NCLS), np.float32)
    for c in range(NCORES):
        mine = np.where(core_of == c)[0]
        out[mine] = results[c]["out"][lr_of[mine]]
    return out


def kernel(**inputs):
    plan = _plan(np.asarray(inputs["edge_index"]))
    nc = _build(plan)
    maps = _in_maps(inputs, plan)
    res = run_bass_kernel_spmd(nc, maps, core_ids=list(range(NCORES)), trace=False)
    return _unpermute(res.results, plan)
